# revision 8
# baseline (speedup 1.0000x reference)
"""Causal multi-head attention (B=4, N=2048, D=768, H=12) on 8 TRN2 cores.

Sharding (per spec hint): data-parallel over batch (4) x tensor-parallel
over heads (2 groups of 6 heads). Core c handles batch c//2, heads
(c%2)*6 .. +6, over the FULL sequence. Host splits w_qkv/w_out
column/row-wise per head group and sums the two partial out-projection
results per batch (row-parallel reduction done host-side).

Per core, fully on-device, no collectives (all bf16 operands, f32 psum):
  qT/kT = Wq/k^T x^T   ([feat, seq] layout, 3 tiles of 2 heads each)
  v     = x Wv         ([seq, feat] layout + ones column for denoms)
  flash-style: per (head, 1024-query window, 128-key tile):
    s = kT_tile^T qT_win (PSUM f32); e = exp(s/8) -> bf16 SBUF
    diagonal tiles: e *= tril-mask (gpsimd)
    oT[65, win] += v_tile^T e  (65th row = softmax denominators)
  aT = oT[0:64] * (1/denom broadcast)   out = aT^T Wo  (partial, f32)

To keep the PE HAM clock-gate warm (the exp on Scalar is the phase
bottleneck; an under-occupied PE re-throttles to 1.2 GHz), projection
and out-projection matmul groups are injected as filler jobs between a
window's score matmuls and the (exp-dependent) attnV matmuls, sharing
the score PSUM pool's rotating buffers.
"""

import numpy as np
from collections import deque

B, N, D, H = 4, 2048, 768, 12
DH = 64            # head dim
HPC = 6            # heads per core
FB = 3             # feature blocks (2 heads = 128 feats) per core
KC = 6             # contraction chunks (768 / 128)
JT = 16            # key tiles (2048 / 128)
QW = 1024          # query window
NQW = N // QW      # 2 windows
G = HPC * DH       # 384 features per core
VW = HPC * (DH + 1)  # 390 v-columns per key tile (with ones)

_CACHE = {}


def _build_nc(mm_dt_name="bfloat16"):
    import concourse.bacc as bacc
    import concourse.mybir as mybir
    import concourse.tile as tile
    import concourse.bass as cbass
    from contextlib import ExitStack

    dt = mybir.dt
    f32 = dt.float32
    bf16 = dt.bfloat16
    Exp = mybir.ActivationFunctionType.Exp

    nc = bacc.Bacc(None)
    xt = nc.declare_dram_parameter("xt", [D, N], bf16, isOutput=False)
    wq = nc.declare_dram_parameter("wq", [D, G], bf16, isOutput=False)
    wk = nc.declare_dram_parameter("wk", [D, G], bf16, isOutput=False)
    wv = nc.declare_dram_parameter("wv", [D, G], bf16, isOutput=False)
    wo = nc.declare_dram_parameter("wo", [G, D], bf16, isOutput=False)
    msk = nc.declare_dram_parameter("msk", [128, 128], bf16, isOutput=False)
    o = nc.declare_dram_parameter("o", [N, D], f32, isOutput=True)

    with tile.TileContext(nc) as tc:
        with ExitStack() as es:
            persist = es.enter_context(tc.tile_pool(name="persist", bufs=1))
            xts = persist.tile([128, KC * N], bf16, tag="xts", name="xts")
            wqs = persist.tile([128, KC * G], bf16, tag="wqs", name="wqs")
            wks = persist.tile([128, KC * G], bf16, tag="wks", name="wks")
            wvs = persist.tile([128, KC * G], bf16, tag="wvs", name="wvs")
            wos = persist.tile([128, FB * D], bf16, tag="wos", name="wos")
            qT = [persist.tile([128, N], bf16, tag=f"qT{f}", name=f"qT{f}")
                  for f in range(FB)]
            kT = [persist.tile([128, N], bf16, tag=f"kT{f}", name=f"kT{f}")
                  for f in range(FB)]
            aT = [persist.tile([128, N], bf16, tag=f"aT{f}", name=f"aT{f}")
                  for f in range(FB)]
            vsb = persist.tile([128, JT * VW], bf16, tag="vsb", name="vsb")
            mskt = persist.tile([128, 128], bf16, tag="mskt", name="mskt")
            pden = persist.tile([128, HPC * NQW * 8], f32, tag="pden",
                                name="pden")
            prec = persist.tile([128, HPC * NQW * 8], f32, tag="prec",
                                name="prec")

            nc.sync.dma_start(out=mskt[:], in_=msk[:, :])
            for c in range(KC):
                nc.sync.dma_start(out=xts[:, c * N:(c + 1) * N],
                                  in_=xt[c * 128:(c + 1) * 128, :])
            for (w_sb, w_dr) in ((wqs, wq), (wks, wk), (wvs, wv)):
                for c in range(KC):
                    nc.sync.dma_start(out=w_sb[:, c * G:(c + 1) * G],
                                      in_=w_dr[c * 128:(c + 1) * 128, :])
            for p in range(FB):
                nc.sync.dma_start(out=wos[:, p * D:(p + 1) * D],
                                  in_=wo[p * 128:(p + 1) * 128, :])

            vv_all = vsb.rearrange("p (j c) -> p j c", c=DH + 1)
            nc.vector.memset(vv_all[:, :, DH:DH + 1], 1.0)

            with tc.tile_pool(name="pss", bufs=2, space="PSUM") as pss, \
                 tc.tile_pool(name="pso", bufs=2, space="PSUM") as pso, \
                 tc.tile_pool(name="et", bufs=4) as etp, \
                 tc.tile_pool(name="dt", bufs=2) as dtp, \
                 tc.tile_pool(name="tb", bufs=2) as tbp, \
                 tc.tile_pool(name="rb", bufs=2) as rbp, \
                 tc.tile_pool(name="ob", bufs=3) as obp, \
                 tc.tile_pool(name="dr", bufs=1, space="DRAM") as drp:
                recd = drp.tile([HPC, N], f32, tag="recd", name="recd")

                # ---------- filler jobs (each: one PSUM group on pss) ----
                def v_job(kb):
                    def run():
                        ps = pss.tile([128, QW], f32, tag="ps", name="ps")
                        for c in range(KC):
                            nc.tensor.matmul(
                                out=ps[:, 0:G],
                                lhsT=xts[:, c * N + kb * 128:
                                         c * N + (kb + 1) * 128],
                                rhs=wvs[:, c * G:(c + 1) * G],
                                start=(c == 0), stop=(c == KC - 1),
                                skip_group_check=True)
                        dst = vsb[:, kb * VW:(kb + 1) * VW].rearrange(
                            "p (h c) -> p h c", c=DH + 1)
                        nc.vector.tensor_copy(
                            dst[:, :, 0:DH],
                            ps[:, 0:G].rearrange("p (h c) -> p h c", c=DH))
                    return run

                def qk_job(w_sb, dstT, fb, half):
                    def run():
                        ps = pss.tile([128, QW], f32, tag="ps", name="ps")
                        for c in range(KC):
                            for q2 in range(2):
                                nc.tensor.matmul(
                                    out=ps[:, q2 * 512:(q2 + 1) * 512],
                                    lhsT=w_sb[:, c * G + fb * 128:
                                              c * G + (fb + 1) * 128],
                                    rhs=xts[:, c * N + half * QW + q2 * 512:
                                            c * N + half * QW + (q2 + 1) * 512],
                                    start=(c == 0), stop=(c == KC - 1),
                                    skip_group_check=True)
                        nc.vector.tensor_copy(
                            dstT[fb][:, half * QW:(half + 1) * QW], ps[:])
                    return run

                def out_job(ib):
                    def run():
                        ps = pss.tile([128, QW], f32, tag="ps", name="ps")
                        for p in range(FB):
                            for (a, b_) in ((0, 512), (512, D)):
                                nc.tensor.matmul(
                                    out=ps[:, a:b_],
                                    lhsT=aT[p][:, ib * 128:(ib + 1) * 128],
                                    rhs=wos[:, p * D + a:p * D + b_],
                                    start=(p == 0), stop=(p == FB - 1),
                                    skip_group_check=True)
                        ot = obp.tile([128, D], f32, tag="ot", name="ot")
                        nc.vector.tensor_copy(ot[:], ps[:, 0:D])
                        nc.sync.dma_start(out=o[ib * 128:(ib + 1) * 128, :],
                                          in_=ot[:])
                    return run

                jobq = deque()

                def drain(n=None):
                    k = len(jobq) if n is None else min(n, len(jobq))
                    for _ in range(k):
                        jobq.popleft()()

                # upfront: q/k for heads 0/1, v for key tiles 0..7
                for half in range(NQW):
                    qk_job(wqs, qT, 0, half)()
                    qk_job(wks, kT, 0, half)()
                for kb in range(8):
                    v_job(kb)()
                # deferred fillers, injected into h0/h1 attention
                for kb in range(8, JT):
                    jobq.append(v_job(kb))
                for fb in range(1, FB):
                    for half in range(NQW):
                        jobq.append(qk_job(wqs, qT, fb, half))
                        jobq.append(qk_job(wks, kT, fb, half))

                # ---------- attention ----------
                def normalize(h, qb, fb, hs):
                    col = (h * NQW + qb) * 8
                    nc.vector.reciprocal(out=prec[:, col:col + 8],
                                         in_=pden[:, col:col + 8])
                    dst = recd[h:h + 1, qb * QW:(qb + 1) * QW].rearrange(
                        "h (p j) -> h p j", j=8)
                    nc.sync.dma_start(out=dst, in_=prec[:, col:col + 8])
                    rbt = rbp.tile([128, QW], bf16, tag="rb", name="rb")
                    src = recd[h:h + 1, qb * QW:(qb + 1) * QW]
                    bcast = cbass.AP(tensor=src.tensor, offset=src.offset,
                                     ap=[[0, 64]] + [list(a) for a in
                                                     src.ap[1:]])
                    nc.gpsimd.dma_start(out=rbt[hs, :], in_=bcast)
                    nc.vector.tensor_mul(
                        aT[fb][hs, qb * QW:(qb + 1) * QW],
                        aT[fb][hs, qb * QW:(qb + 1) * QW], rbt[hs, :])

                for h in range(HPC):
                    fb, hh = h // 2, h % 2
                    hs = slice(hh * 64, hh * 64 + 64)
                    if h == 2:
                        drain()  # safety: fb1 q/k must be in SBUF
                    if h == 4:
                        drain()
                    for qb in range(NQW):
                        ntile = 8 * qb + 8
                        qc0 = qb * QW
                        if h == HPC - 1 and qb == 1:
                            for ib in range(8):
                                jobq.append(out_job(ib))
                        oT = pso.tile([128, QW], f32, tag="oT", name="oT")

                        def chunks(c0):
                            if c0 < 512:
                                return ((c0, 512), (512, QW))
                            return ((c0, QW),)

                        def attnv(pjt, pet, pc0, first, last):
                            vsl = vsb[:, pjt * VW + h * (DH + 1):
                                      pjt * VW + (h + 1) * (DH + 1)]
                            for (a, b_) in chunks(pc0):
                                nc.tensor.matmul(
                                    out=oT[0:DH + 1, a:b_],
                                    lhsT=vsl, rhs=pet[:, a:b_],
                                    start=first, stop=last,
                                    skip_group_check=True)

                        prev = None
                        for jt in range(ntile):
                            c0 = max(0, (jt - 8 * qb) * 128)
                            ps = pss.tile([128, QW], f32, tag="ps", name="ps")
                            for (a, b_) in chunks(c0):
                                nc.tensor.matmul(
                                    out=ps[:, a:b_],
                                    lhsT=kT[fb][hs, jt * 128:(jt + 1) * 128],
                                    rhs=qT[fb][hs, qc0 + a:qc0 + b_],
                                    start=True, stop=True,
                                    skip_group_check=True)
                            if jt % 2 == 1:
                                drain(1)  # PE filler while exp runs
                            if prev is not None:
                                attnv(prev[0], prev[1], prev[2],
                                      prev[0] == 0, False)
                            et = etp.tile([128, QW], bf16, tag="et", name="et")
                            nc.scalar.activation(out=et[:, c0:QW],
                                                 in_=ps[:, c0:QW],
                                                 func=Exp, scale=0.125)
                            if jt >= 8 * qb:
                                nc.gpsimd.tensor_mul(
                                    et[:, c0:c0 + 128],
                                    et[:, c0:c0 + 128], mskt[:])
                            prev = (jt, et, c0)
                        attnv(prev[0], prev[1], prev[2], False, True)

                        # evacuate window: features -> aT, denoms -> pden
                        dtt = dtp.tile([DH + 1, QW], f32, tag="dt", name="dt")
                        if hh == 0:
                            nc.vector.tensor_copy(
                                aT[fb][0:DH, qc0:qc0 + QW], oT[0:DH, :])
                        else:
                            tbt = tbp.tile([DH, QW], bf16, tag="tb", name="tb")
                            nc.vector.tensor_copy(tbt[:], oT[0:DH, :])
                            nc.sync.dma_start(
                                out=aT[fb][DH:2 * DH, qc0:qc0 + QW],
                                in_=tbt[:])
                        nc.vector.tensor_copy(dtt[DH:DH + 1, :],
                                              oT[DH:DH + 1, :])
                        col = (h * NQW + qb) * 8
                        nc.sync.dma_start(out=pden[:, col:col + 8],
                                          in_=dtt[DH:DH + 1, :])
                        normalize(h, qb, fb, hs)

                # ---------- drain remaining output projection ----------
                drain()
                for ib in range(8, N // 128):
                    out_job(ib)()

    nc.finalize()
    return nc


def _host_reference(x, mask, w_qkv, w_out):
    qkv = x.astype(np.float64) @ w_qkv.astype(np.float64)
    q, k, v = np.split(qkv, 3, axis=-1)

    def heads(t):
        return t.reshape(B, N, H, DH).transpose(0, 2, 1, 3)
    q, k, v = heads(q), heads(k), heads(v)
    s = np.einsum('bhqd,bhkd->bhqk', q, k) / np.sqrt(DH)
    s = np.where(np.asarray(mask).reshape(1, 1, N, N) == 0, -np.inf, s)
    s = s - s.max(-1, keepdims=True)
    e = np.exp(s)
    p = e / e.sum(-1, keepdims=True)
    out = np.einsum('bhqk,bhkd->bhqd', p, v)
    out = out.transpose(0, 2, 1, 3).reshape(B, N, D)
    return (out @ w_out.astype(np.float64)).astype(np.float32)


def kernel(x, mask, w_qkv, w_out):
    import ml_dtypes
    bf = ml_dtypes.bfloat16
    x = np.asarray(x)
    w_qkv = np.asarray(w_qkv)
    w_out = np.asarray(w_out)

    causal = np.array_equal(
        np.asarray(mask).reshape(N, N) != 0, np.tril(np.ones((N, N), bool)))
    if not causal:
        return _host_reference(x, mask, w_qkv, w_out)

    from concourse.bass_utils import run_bass_kernel_spmd
    if "nc" not in _CACHE:
        _CACHE["nc"] = _build_nc()
    nc = _CACHE["nc"]

    msk_np = np.triu(np.ones((128, 128), np.float32)).astype(bf)
    in_maps = []
    for c in range(8):
        b, g = c // 2, c % 2
        in_maps.append({
            "xt": np.ascontiguousarray(x[b].T).astype(bf),
            "wq": np.ascontiguousarray(
                w_qkv[:, g * G:(g + 1) * G]).astype(bf),
            "wk": np.ascontiguousarray(
                w_qkv[:, D + g * G:D + (g + 1) * G]).astype(bf),
            "wv": np.ascontiguousarray(
                w_qkv[:, 2 * D + g * G:2 * D + (g + 1) * G]).astype(bf),
            "wo": np.ascontiguousarray(
                w_out[g * G:(g + 1) * G, :]).astype(bf),
            "msk": msk_np,
        })
    res = run_bass_kernel_spmd(nc, in_maps, core_ids=list(range(8)),
                               **_CACHE.get("run_kwargs", {}))
    _CACHE["last_res"] = res
    out = np.empty((B, N, D), np.float32)
    for b in range(B):
        out[b] = res.results[2 * b]["o"]
        out[b] += res.results[2 * b + 1]["o"]
    return out


# revision 15
# speedup vs baseline: 1.2365x; 1.2365x over previous
"""Causal multi-head attention (B=4, N=2048, D=768, H=12) on 8 TRN2 cores.

Sharding (per spec hint): data-parallel over batch (4) x tensor-parallel
over heads (2 groups of 6 heads). Core c handles batch c//2, heads
(c%2)*6 .. +6, over the FULL sequence. Host splits w_qkv/w_out
column/row-wise per head group and sums the two partial out-projection
results per batch (row-parallel reduction done host-side).

Per core, fully on-device, no collectives (all bf16 operands, f32 psum):
  qT/kT = Wq/k^T x^T   ([feat, seq] layout, 3 tiles of 2 heads each)
  v     = x Wv         ([seq, feat] layout + ones column for denoms)
  flash-style: per (head, 1024-query window, 128-key tile):
    s = kT_tile^T qT_win (PSUM f32); e = exp(s/8) -> bf16 SBUF
    diagonal tiles: e *= tril-mask (gpsimd)
    oT[65, win] += v_tile^T e  (65th row = softmax denominators)
  aT = oT[0:64] * (1/denom broadcast)   out = aT^T Wo  (partial, f32)

To keep the PE HAM clock-gate warm (the exp on Scalar is the phase
bottleneck; an under-occupied PE re-throttles to 1.2 GHz), projection
and out-projection matmul groups are injected as filler jobs between a
window's score matmuls and the (exp-dependent) attnV matmuls, sharing
the score PSUM pool's rotating buffers.
"""

import numpy as np
from collections import deque

B, N, D, H = 4, 2048, 768, 12
DH = 64            # head dim
HPC = 6            # heads per core
FB = 3             # feature blocks (2 heads = 128 feats) per core
KC = 6             # contraction chunks (768 / 128)
JT = 16            # key tiles (2048 / 128)
QW = 1024          # query window
NQW = N // QW      # 2 windows
G = HPC * DH       # 384 features per core
VW = HPC * (DH + 1)  # 390 v-columns per key tile (with ones)

_CACHE = {}


def _build_nc(mm_dt_name="bfloat16"):
    import concourse.bacc as bacc
    import concourse.mybir as mybir
    import concourse.tile as tile
    import concourse.bass as cbass
    from contextlib import ExitStack

    dt = mybir.dt
    f32 = dt.float32
    bf16 = dt.bfloat16
    Exp = mybir.ActivationFunctionType.Exp

    nc = bacc.Bacc(None)
    xt = nc.declare_dram_parameter("xt", [D, N], bf16, isOutput=False)
    wq = nc.declare_dram_parameter("wq", [D, G], bf16, isOutput=False)
    wk = nc.declare_dram_parameter("wk", [D, G], bf16, isOutput=False)
    wv = nc.declare_dram_parameter("wv", [D, G], bf16, isOutput=False)
    wo = nc.declare_dram_parameter("wo", [G, D], bf16, isOutput=False)
    msk = nc.declare_dram_parameter("msk", [128, 128], bf16, isOutput=False)
    o = nc.declare_dram_parameter("o", [N, D], f32, isOutput=True)

    with tile.TileContext(nc) as tc:
        with ExitStack() as es:
            persist = es.enter_context(tc.tile_pool(name="persist", bufs=1))
            xts = persist.tile([128, KC * N], bf16, tag="xts", name="xts")
            wqs = persist.tile([128, KC * G], bf16, tag="wqs", name="wqs")
            wks = persist.tile([128, KC * G], bf16, tag="wks", name="wks")
            wvs = persist.tile([128, KC * G], bf16, tag="wvs", name="wvs")
            wos = persist.tile([128, FB * D], bf16, tag="wos", name="wos")
            qT = [persist.tile([128, N], bf16, tag=f"qT{f}", name=f"qT{f}")
                  for f in range(FB)]
            kT = [persist.tile([128, N], bf16, tag=f"kT{f}", name=f"kT{f}")
                  for f in range(FB)]
            aT = [persist.tile([128, N], bf16, tag=f"aT{f}", name=f"aT{f}")
                  for f in range(FB)]
            vsb = persist.tile([128, JT * VW], bf16, tag="vsb", name="vsb")
            osum = persist.tile([128, (N // 128) * D], f32, tag="osum",
                                name="osum")
            mskt = persist.tile([128, 128], bf16, tag="mskt", name="mskt")
            pden = persist.tile([128, HPC * NQW * 8], f32, tag="pden",
                                name="pden")
            prec = persist.tile([128, HPC * NQW * 8], f32, tag="prec",
                                name="prec")

            nc.sync.dma_start(out=mskt[:], in_=msk[:, :])
            for c in range(KC):
                nc.sync.dma_start(out=xts[:, c * N:(c + 1) * N],
                                  in_=xt[c * 128:(c + 1) * 128, :])
            for (w_sb, w_dr) in ((wqs, wq), (wks, wk), (wvs, wv)):
                for c in range(KC):
                    nc.sync.dma_start(out=w_sb[:, c * G:(c + 1) * G],
                                      in_=w_dr[c * 128:(c + 1) * 128, :])
            for p in range(FB):
                nc.sync.dma_start(out=wos[:, p * D:(p + 1) * D],
                                  in_=wo[p * 128:(p + 1) * 128, :])

            vv_all = vsb.rearrange("p (j c) -> p j c", c=DH + 1)
            nc.vector.memset(vv_all[:, :, DH:DH + 1], 1.0)

            with tc.tile_pool(name="pss", bufs=2, space="PSUM") as pss, \
                 tc.tile_pool(name="pso", bufs=2, space="PSUM") as pso, \
                 tc.tile_pool(name="et", bufs=3) as etp, \
                 tc.tile_pool(name="dt", bufs=2) as dtp, \
                 tc.tile_pool(name="tb", bufs=2) as tbp, \
                 tc.tile_pool(name="rb", bufs=2) as rbp, \
                 tc.tile_pool(name="ob", bufs=3) as obp, \
                 tc.tile_pool(name="dr", bufs=1, space="DRAM") as drp:
                recd = drp.tile([HPC, N], f32, tag="recd", name="recd")

                # ---------- filler jobs (each: one PSUM group on pss) ----
                def v_job(kb):
                    def run():
                        ps = pss.tile([128, QW], f32, tag="ps", name="ps")
                        for c in range(KC):
                            nc.tensor.matmul(
                                out=ps[:, 0:G],
                                lhsT=xts[:, c * N + kb * 128:
                                         c * N + (kb + 1) * 128],
                                rhs=wvs[:, c * G:(c + 1) * G],
                                start=(c == 0), stop=(c == KC - 1),
                                skip_group_check=True)
                        dst = vsb[:, kb * VW:(kb + 1) * VW].rearrange(
                            "p (h c) -> p h c", c=DH + 1)
                        nc.vector.tensor_copy(
                            dst[:, :, 0:DH],
                            ps[:, 0:G].rearrange("p (h c) -> p h c", c=DH))
                    return run

                def qk_job(w_sb, dstT, fb, half):
                    def run():
                        ps = pss.tile([128, QW], f32, tag="ps", name="ps")
                        for c in range(KC):
                            for q2 in range(2):
                                nc.tensor.matmul(
                                    out=ps[:, q2 * 512:(q2 + 1) * 512],
                                    lhsT=w_sb[:, c * G + fb * 128:
                                              c * G + (fb + 1) * 128],
                                    rhs=xts[:, c * N + half * QW + q2 * 512:
                                            c * N + half * QW + (q2 + 1) * 512],
                                    start=(c == 0), stop=(c == KC - 1),
                                    skip_group_check=True)
                        nc.vector.tensor_copy(
                            dstT[fb][:, half * QW:(half + 1) * QW], ps[:])
                    return run

                def out_job(p, ib):
                    def run():
                        ps = pss.tile([128, QW], f32, tag="ps", name="ps")
                        for (a, b_) in ((0, 512), (512, D)):
                            nc.tensor.matmul(
                                out=ps[:, a:b_],
                                lhsT=aT[p][:, ib * 128:(ib + 1) * 128],
                                rhs=wos[:, p * D + a:p * D + b_],
                                start=True, stop=True,
                                skip_group_check=True)
                        osl = osum[:, ib * D:(ib + 1) * D]
                        if p == 0:
                            nc.vector.tensor_copy(osl, ps[:, 0:D])
                        else:
                            nc.vector.scalar_tensor_tensor(
                                out=osl, in0=ps[:, 0:D], scalar=1.0, in1=osl,
                                op0=mybir.AluOpType.mult,
                                op1=mybir.AluOpType.add)
                        if p == FB - 1:
                            nc.sync.dma_start(
                                out=o[ib * 128:(ib + 1) * 128, :], in_=osl)
                    return run

                jobq = deque()

                def drain(n=None):
                    k = len(jobq) if n is None else min(n, len(jobq))
                    for _ in range(k):
                        jobq.popleft()()

                # upfront: q/k for heads 0/1, first v tile
                for half in range(NQW):
                    qk_job(wqs, qT, 0, half)()
                    qk_job(wks, kT, 0, half)()
                v_job(0)()
                # deferred fillers, injected into the attention PE stream
                for kb in range(1, JT):
                    jobq.append(v_job(kb))
                for fb in range(1, 2):
                    for half in range(NQW):
                        jobq.append(qk_job(wqs, qT, fb, half))
                        jobq.append(qk_job(wks, kT, fb, half))

                # ---------- attention ----------
                def normalize(h, qb, fb, hs):
                    col = (h * NQW + qb) * 8
                    nc.vector.reciprocal(out=prec[:, col:col + 8],
                                         in_=pden[:, col:col + 8])
                    dst = recd[h:h + 1, qb * QW:(qb + 1) * QW].rearrange(
                        "h (p j) -> h p j", j=8)
                    nc.sync.dma_start(out=dst, in_=prec[:, col:col + 8])
                    rbt = rbp.tile([128, QW], bf16, tag="rb", name="rb")
                    src = recd[h:h + 1, qb * QW:(qb + 1) * QW]
                    bcast = cbass.AP(tensor=src.tensor, offset=src.offset,
                                     ap=[[0, 64]] + [list(a) for a in
                                                     src.ap[1:]])
                    nc.gpsimd.dma_start(out=rbt[hs, :], in_=bcast)
                    nc.vector.tensor_mul(
                        aT[fb][hs, qb * QW:(qb + 1) * QW],
                        aT[fb][hs, qb * QW:(qb + 1) * QW], rbt[hs, :])

                for h in range(HPC):
                    fb, hh = h // 2, h % 2
                    hs = slice(hh * 64, hh * 64 + 64)
                    if h == 2:
                        for half in range(NQW):
                            jobq.append(qk_job(wqs, qT, 2, half))
                            jobq.append(qk_job(wks, kT, 2, half))
                        for ib in range(N // 128):
                            jobq.append(out_job(0, ib))
                    if h == 4:
                        for ib in range(N // 128):
                            jobq.append(out_job(1, ib))
                    for qb in range(NQW):
                        ntile = 8 * qb + 8
                        qc0 = qb * QW
                        cad = (1 if qb == 0 else 2) if h == 0 else \
                            4 if h == 1 else 2 if h in (2, 3) else 3
                        oT = pso.tile([128, QW], f32, tag="oT", name="oT")

                        def chunks(c0):
                            if c0 < 512:
                                return ((c0, 512), (512, QW))
                            return ((c0, QW),)

                        def attnv(pjt, pet, pc0, first, last):
                            vsl = vsb[:, pjt * VW + h * (DH + 1):
                                      pjt * VW + (h + 1) * (DH + 1)]
                            for (a, b_) in chunks(pc0):
                                nc.tensor.matmul(
                                    out=oT[0:DH + 1, a:b_],
                                    lhsT=vsl, rhs=pet[:, a:b_],
                                    start=first, stop=last,
                                    skip_group_check=True)

                        prev = None
                        for jt in range(ntile):
                            c0 = max(0, (jt - 8 * qb) * 128)
                            ps = pss.tile([128, QW], f32, tag="ps", name="ps")
                            for (a, b_) in chunks(c0):
                                nc.tensor.matmul(
                                    out=ps[:, a:b_],
                                    lhsT=kT[fb][hs, jt * 128:(jt + 1) * 128],
                                    rhs=qT[fb][hs, qc0 + a:qc0 + b_],
                                    start=True, stop=True,
                                    skip_group_check=True)
                            if jt % cad == cad - 1:
                                drain(1)  # PE filler while exp runs
                            if prev is not None:
                                attnv(prev[0], prev[1], prev[2],
                                      prev[0] == 0, False)
                            et = etp.tile([128, QW], bf16, tag="et", name="et")
                            nc.scalar.activation(out=et[:, c0:QW],
                                                 in_=ps[:, c0:QW],
                                                 func=Exp, scale=0.125)
                            if jt >= 8 * qb:
                                nc.gpsimd.tensor_mul(
                                    et[:, c0:c0 + 128],
                                    et[:, c0:c0 + 128], mskt[:])
                            prev = (jt, et, c0)
                        attnv(prev[0], prev[1], prev[2], False, True)

                        # evacuate window: features -> aT, denoms -> pden
                        dtt = dtp.tile([DH + 1, QW], f32, tag="dt", name="dt")
                        if hh == 0:
                            nc.vector.tensor_copy(
                                aT[fb][0:DH, qc0:qc0 + QW], oT[0:DH, :])
                        else:
                            tbt = tbp.tile([DH, QW], bf16, tag="tb", name="tb")
                            nc.vector.tensor_copy(tbt[:], oT[0:DH, :])
                            nc.sync.dma_start(
                                out=aT[fb][DH:2 * DH, qc0:qc0 + QW],
                                in_=tbt[:])
                        nc.vector.tensor_copy(dtt[DH:DH + 1, :],
                                              oT[DH:DH + 1, :])
                        col = (h * NQW + qb) * 8
                        nc.sync.dma_start(out=pden[:, col:col + 8],
                                          in_=dtt[DH:DH + 1, :])
                        normalize(h, qb, fb, hs)

                # ---------- drain queue + final out-projection pair ----------
                drain()
                for ib in range(N // 128):
                    out_job(FB - 1, ib)()

    nc.finalize()
    return nc


def _host_reference(x, mask, w_qkv, w_out):
    qkv = x.astype(np.float64) @ w_qkv.astype(np.float64)
    q, k, v = np.split(qkv, 3, axis=-1)

    def heads(t):
        return t.reshape(B, N, H, DH).transpose(0, 2, 1, 3)
    q, k, v = heads(q), heads(k), heads(v)
    s = np.einsum('bhqd,bhkd->bhqk', q, k) / np.sqrt(DH)
    s = np.where(np.asarray(mask).reshape(1, 1, N, N) == 0, -np.inf, s)
    s = s - s.max(-1, keepdims=True)
    e = np.exp(s)
    p = e / e.sum(-1, keepdims=True)
    out = np.einsum('bhqk,bhkd->bhqd', p, v)
    out = out.transpose(0, 2, 1, 3).reshape(B, N, D)
    return (out @ w_out.astype(np.float64)).astype(np.float32)


def kernel(x, mask, w_qkv, w_out):
    import ml_dtypes
    bf = ml_dtypes.bfloat16
    x = np.asarray(x)
    w_qkv = np.asarray(w_qkv)
    w_out = np.asarray(w_out)

    causal = np.array_equal(
        np.asarray(mask).reshape(N, N) != 0, np.tril(np.ones((N, N), bool)))
    if not causal:
        return _host_reference(x, mask, w_qkv, w_out)

    from concourse.bass_utils import run_bass_kernel_spmd
    if "nc" not in _CACHE:
        _CACHE["nc"] = _build_nc()
    nc = _CACHE["nc"]

    msk_np = np.triu(np.ones((128, 128), np.float32)).astype(bf)
    in_maps = []
    for c in range(8):
        b, g = c // 2, c % 2
        in_maps.append({
            "xt": np.ascontiguousarray(x[b].T).astype(bf),
            "wq": np.ascontiguousarray(
                w_qkv[:, g * G:(g + 1) * G]).astype(bf),
            "wk": np.ascontiguousarray(
                w_qkv[:, D + g * G:D + (g + 1) * G]).astype(bf),
            "wv": np.ascontiguousarray(
                w_qkv[:, 2 * D + g * G:2 * D + (g + 1) * G]).astype(bf),
            "wo": np.ascontiguousarray(
                w_out[g * G:(g + 1) * G, :]).astype(bf),
            "msk": msk_np,
        })
    res = run_bass_kernel_spmd(nc, in_maps, core_ids=list(range(8)),
                               **_CACHE.get("run_kwargs", {}))
    _CACHE["last_res"] = res
    out = np.empty((B, N, D), np.float32)
    for b in range(B):
        out[b] = res.results[2 * b]["o"]
        out[b] += res.results[2 * b + 1]["o"]
    return out
